# revision 46
# baseline (speedup 1.0000x reference)
"""Causal multi-head attention block on 8 Trainium2 NeuronCores.

Problem: x[4,2048,1024] -> QKV proj (16 heads, dh=64) -> causal softmax
attention -> out proj. Sharding: core = (batch, head-half): each core
computes QKV for 8 heads of one batch, flash-style attention for those
heads, and a partial O-projection over its 512 W_o input columns; the
host sums the two partials per batch (tensor-parallel unshard).

Device kernel (identical SPMD program, per-core data), matmuls bf16
with fp32 PSUM accumulation, EXCEPT the Q/K projection which runs in
fp8e4m3 with DoubleRow perf mode: contraction 1024 in 4 passes instead
of 8 (DoubleRow processes 2 k-subtiles per pass), a true 2x on the PE.
x and W_q/W_k are pre-scaled (x16 / x256) on the host so fp8
quantization stays clear of subnormals; Q.T/K.T are stored bf16 SCALED
by 4096 and the 1/4096^2 descale is folded into the exp's activation
scale. V projection, scores, P@V and O-projection stay bf16 (fp8
there fails the accuracy budget; fp8 DoubleRow on the K=64 score
matmuls gives no wall-clock win anyway - the win scales with
contraction-pass count).

  - x.T is host pre-transposed; Q.T/K.T computed in [o, t] feature-major
    layout, V in [t, o].
  - scores are computed transposed, S.T[k_tile, q_span] = K.T_blk^T@Q.T,
    two k-tiles packed side by side in one 2-bank PSUM tile so ScalarE
    exps them in a single ACTIVATE (descale folded in; scores are O(1)
    here so softmax needs no max-subtraction). Diagonal blocks are
    masked after exp with a 0/1 triangle multiply on the Pool engine
    (GpSimd), keeping DVE free for PSUM evictions.
  - O.T[c, q] accumulates with V' stationary: V' has 64 V columns and 64
    ones-columns (parity-dependent order), so each head's O.T lands on
    its final partition rows and the matmul broadcasts the softmax
    denominator into the other 64 rows for free. Normalization is then
    one reciprocal_approx_fast + one cross-partition-base multiply
    (odd heads read the denominator straight from PSUM partition 0,
    even heads need one repositioning copy), writing OT[c, t] directly.
  - two heads are software-pipelined (PE runs head B scores while
    ScalarE exps head A) and the P@V matmuls lag one iteration behind
    the exps; QKV-projection and O-projection units are paced into the
    attention phase as PE filler so the ScalarE-bound stretches keep the
    TensorE array busy (and the HAM clock-gate warm). A short burst of
    junk matmuls warms the PE while the input DMAs stream; inputs are
    DMA'd in consumption order across three rings so the first
    projection unit starts ~2us in.
"""

import numpy as np
import ml_dtypes

BF16 = ml_dtypes.bfloat16
E4M3 = ml_dtypes.float8_e4m3

B, T, D = 4, 2048, 1024
NH, DH = 16, 64
HPC = 8            # heads per core
OC = HPC * DH      # 512: per-core head columns
NT = T // 128      # 16 q/k tiles of 128
ND = D // 128      # 8 d-tiles
N_CORES = 8
SX = 16.0          # fp8 x pre-scale
SW = 256.0         # fp8 W_q/W_k pre-scale

_cache = {}


def _build(debug=False):
    import concourse.mybir as mybir
    import concourse.tile as tile
    from concourse import bacc

    f32 = mybir.dt.float32
    bf16 = mybir.dt.bfloat16
    f8 = mybir.dt.float8e4
    Exp = mybir.ActivationFunctionType.Exp
    DR = mybir.MatmulPerfMode.DoubleRow

    nc = bacc.Bacc("TRN2", target_bir_lowering=False, debug=False,
                   num_devices=N_CORES)

    # all large inputs are host-pre-tiled to [128, ...] partition-major
    # layout so every DMA is one contiguous multi-KB run per partition
    xT = nc.declare_dram_parameter("xT", [128, ND * T], bf16, isOutput=False)
    x8 = nc.declare_dram_parameter("x8", [128, ND * T], f8, isOutput=False)
    w8 = nc.declare_dram_parameter("w8", [128, ND * 2 * OC], f8, isOutput=False)
    wv = nc.declare_dram_parameter("wvT", [128, ND * OC], bf16, isOutput=False)
    wo = nc.declare_dram_parameter("woT", [128, (OC // 128) * D], bf16,
                                   isOutput=False)
    bqk = nc.declare_dram_parameter("bqk", [128, 2 * OC // 128], f32, isOutput=False)
    bv = nc.declare_dram_parameter("bv", [1, OC], f32, isOutput=False)
    bo = nc.declare_dram_parameter("bo", [1, D], f32, isOutput=False)
    tri = nc.declare_dram_parameter("tri", [128, 128], bf16, isOutput=False)
    # bf16 wire format: halves output DMA traffic; host upcasts and sums
    out = nc.declare_dram_parameter("out", [T, D], bf16, isOutput=True)
    if debug:
        d_qkt = nc.declare_dram_parameter("d_qkt", [128, ND * T], bf16, isOutput=True)
        d_ot = nc.declare_dram_parameter(
            "d_ot", [128, (OC // 128) * T], bf16, isOutput=True)

    with tile.TileContext(nc) as tc:
        with (
            tc.tile_pool(name="persist", bufs=1) as persist,
            tc.tile_pool(name="pt", bufs=8) as ptp,
            tc.tile_pool(name="dn", bufs=6) as dnp,
            tc.tile_pool(name="ostage", bufs=4) as ostage,
            tc.tile_pool(name="psS", bufs=3, space="PSUM") as psS,
            tc.tile_pool(name="psO", bufs=2, space="PSUM") as psO,
        ):
            # ---- persistent SBUF tensors ----
            # x.T in t-major layouts so DMA slices arrive in consumption
            # order as contiguous multi-KB runs: XT by 128-t-tile (for
            # emit_v lhsT), X8 by 512-t-quarter (for emit_qk rhs)
            XT = persist.tile([128, NT, ND, 128], bf16)
            X8 = persist.tile([128, 4, ND, 512], f8)
            # W_qk.T fp8, chunk-interleaved [q0,k0,q1,k1,q2,k2,q3,k3] so
            # the prologue pair (q0,k0) is one contiguous leading slice
            W8 = persist.tile([128, 8, ND, 128], f8)
            WV = persist.tile([128, ND, OC], bf16)
            WO = persist.tile([128, OC // 128, D], bf16)
            BQK = persist.tile([128, 2 * OC // 128], f32)
            BV = persist.tile([128, OC], f32)
            BO = persist.tile([128, D], f32)
            TRI = persist.tile([128, 128], bf16)
            QKT = persist.tile([128, ND, T], bf16)         # [o, t] Q.T|K.T
            # V' per head, 128 cols: even h: [V(64) | 1*64]; odd h:
            # [1*64 | V(64)]. O.T rows land on partitions (h%2)*64..+64 and
            # the other 64 rows all become the softmax denominator (the
            # matmul broadcasts it for free).
            VP = persist.tile([128, NT, HPC, 128], bf16)
            OT = persist.tile([128, OC // 128, T], bf16)   # attn out.T [c, t]

            # warm-up: keep PE busy (and the HAM un-throttled) while the
            # input DMAs stream in; results are never read.
            JNK = persist.tile([128, 512], bf16)
            nc.vector.memset(JNK[:], 0.5)
            for g in range(2):
                jps = psS.tile([128, 512], f32, tag="s", name=f"jnk{g}")
                for m in range(11):
                    nc.tensor.matmul(
                        jps[:], lhsT=JNK[:, 0:128], rhs=JNK[:],
                        start=(m == 0), stop=(m == 10),
                    )

            xTr = xT.rearrange("p (a n t) -> p a n t", a=NT, n=ND)
            x8r = x8.rearrange("p (a n t) -> p a n t", a=4, n=ND)
            w8r = w8.rearrange("p (s n o) -> p s n o", s=8, n=ND)
            wvr = wv.rearrange("p (n o) -> p n o", n=ND)
            wor = wo.rearrange("p (c o) -> p c o", c=OC // 128)

            # input DMAs: each launch engine's transfers SERIALIZE on its
            # ring at ~110 GB/s, so tensors are split across the three
            # rings in strict consumption order; host pre-tiling makes
            # every slice one contiguous multi-KB run per partition
            # (fragmented strided DMAs run at <20 GB/s).
            #  sync:   BQK, X8 q0, WV hi, TRI, X8 q2, XT tt8-15 (+outs)
            #  scalar: W8 (q0,k0), WV lo, X8 q1, X8 q3, W8 rest, WO
            #  gpsimd: XT tt0-3, BV, XT tt4-7, BO (+outs)
            nc.sync.dma_start(out=BQK[:], in_=bqk[:, :])
            nc.scalar.dma_start(out=W8[:, 0:2], in_=w8r[:, 0:2])
            nc.gpsimd.dma_start(out=XT[:, 0:4], in_=xTr[:, 0:4])
            nc.sync.dma_start(out=X8[:, 0], in_=x8r[:, 0])
            nc.scalar.dma_start(out=WV[:, 0:4], in_=wvr[:, 0:4])
            nc.gpsimd.dma_start(out=BV[:], in_=bv[:, :].to_broadcast((128, OC)))
            nc.sync.dma_start(out=WV[:, 4:8], in_=wvr[:, 4:8])
            nc.scalar.dma_start(out=X8[:, 1], in_=x8r[:, 1])
            nc.gpsimd.dma_start(out=XT[:, 4:8], in_=xTr[:, 4:8])
            nc.sync.dma_start(out=TRI[:], in_=tri[:, :])
            nc.sync.dma_start(out=X8[:, 2], in_=x8r[:, 2])
            nc.scalar.dma_start(out=X8[:, 3], in_=x8r[:, 3])
            nc.gpsimd.dma_start(out=BO[:], in_=bo[:, :].to_broadcast((128, D)))
            nc.sync.dma_start(out=XT[:, 8:12], in_=xTr[:, 8:12])
            nc.scalar.dma_start(out=W8[:, 2:4], in_=w8r[:, 2:4])
            nc.gpsimd.dma_start(out=XT[:, 12:16], in_=xTr[:, 12:16])
            nc.scalar.dma_start(out=W8[:, 4:8], in_=w8r[:, 4:8])
            nc.scalar.dma_start(out=WO[:, 0:2], in_=wor[:, 0:2])
            nc.scalar.dma_start(out=WO[:, 2:4], in_=wor[:, 2:4])
            # V' ones columns on the Pool engine (SBUF-only op; DVE stays
            # free for evictions)
            nc.gpsimd.memset(VP[:, :, 0:HPC:2, DH:128], 1.0)
            nc.gpsimd.memset(VP[:, :, 1:HPC:2, 0:DH], 1.0)

            # ---- QKV projection, emitted as fill-in units ----
            def emit_qk(ot, tch):
                # one [o, t] chunk: [128 o, 512 t] = W_qk @ x.T + b, in fp8
                # DoubleRow: 4 passes over (kd, kd+1) pairs instead of 8.
                # Result is scaled by SX*SW; bias is pre-scaled to match.
                ps = psS.tile([128, 512], f32, tag="s",
                              name=f"qk{ot}_{tch}")
                ws = 2 * (ot % 4) + (0 if ot < 4 else 1)
                for kd in range(0, ND, 2):
                    nc.tensor.matmul(
                        ps[:],
                        lhsT=W8[:, ws, kd:kd + 2, :],
                        rhs=X8[:, tch, kd:kd + 2, :],
                        start=(kd == 0), stop=(kd == ND - 2),
                        perf_mode=DR,
                    )
                nc.vector.tensor_scalar_add(
                    QKT[:, ot, tch * 512:(tch + 1) * 512], ps[:],
                    BQK[:, ot:ot + 1],
                )

            def emit_v(tt):
                # one [t, o] tile of V = x @ W_v.T + b, into parity layout
                ps = psS.tile([128, 512], f32, tag="s", name=f"v{tt}")
                for kd in range(ND):
                    nc.tensor.matmul(
                        ps[:],
                        lhsT=XT[:, tt, kd, :],
                        rhs=WV[:, kd, :],
                        start=(kd == 0), stop=(kd == ND - 1),
                    )
                nc.vector.tensor_tensor(
                    out=VP[:, tt, 0:HPC:2, 0:DH],
                    in0=ps[:].rearrange("p (a b) -> p a b", b=DH)[:, 0:HPC:2, :],
                    in1=BV[:].rearrange("p (a b) -> p a b", b=DH)[:, 0:HPC:2, :],
                    op=mybir.AluOpType.add,
                )
                nc.vector.tensor_tensor(
                    out=VP[:, tt, 1:HPC:2, DH:2 * DH],
                    in0=ps[:].rearrange("p (a b) -> p a b", b=DH)[:, 1:HPC:2, :],
                    in1=BV[:].rearrange("p (a b) -> p a b", b=DH)[:, 1:HPC:2, :],
                    op=mybir.AluOpType.add,
                )

            # prologue: only what head-pair 0's first iteration needs
            emit_qk(0, 0)
            emit_qk(4, 0)
            # the rest is interleaved into the attention phase as PE
            # filler. Deadlines (giter of the pop_fill that must emit):
            #   Q(m,t): 20m + t^2+t-1; K(4+m,t): 20m + t^2+3t-1
            #   V(i): J=i//4: J^2+J + min(i//2, 2J+1)  (same-giter OK: the
            #   avs of a pair run after that giter's pop_fill)
            sched = {
                0: [("v", 0), ("v", 1)],
                1: [("v", 2), ("v", 3), ("qk", 0, 1)],
                2: [("qk", 4, 1)],
                3: [("v", 4), ("v", 5)],
                4: [("qk", 0, 2), ("v", 6)],
                5: [("v", 7)],
                7: [("qk", 4, 2)],
                8: [("v", 8), ("v", 9)],
                9: [("qk", 0, 3)],
                10: [("v", 10), ("v", 11)],
                14: [("qk", 4, 3)],
                16: [("v", 12), ("v", 13)],
                17: [("v", 14), ("v", 15)],
                18: [("qk", 1, 0)],
                19: [("qk", 5, 0)],
                20: [("qk", 1, 1)],
                22: [("qk", 5, 1)],
                24: [("qk", 1, 2)],
                28: [("qk", 5, 2)],
                30: [("qk", 1, 3)],
                36: [("qk", 5, 3)],
                38: [("qk", 2, 0)],
                39: [("qk", 6, 0)],
                40: [("qk", 2, 1)],
                42: [("qk", 6, 1)],
                44: [("qk", 2, 2)],
                48: [("qk", 6, 2)],
                50: [("qk", 2, 3)],
                56: [("qk", 6, 3)],
                58: [("qk", 3, 0)],
                59: [("qk", 7, 0)],
                60: [("qk", 3, 1)],
                62: [("qk", 7, 1)],
                64: [("qk", 3, 2)],
                68: [("qk", 7, 2)],
                70: [("qk", 3, 3)],
                76: [("qk", 7, 3)],
            }
            giter = [0]
            oproj_q = []

            def pop_fill():
                g = giter[0]
                giter[0] += 1
                for u in sched.get(g, []):
                    if u[0] == "v":
                        emit_v(u[1])
                    else:
                        emit_qk(u[1], u[2])
                for _ in range(3):
                    if oproj_q:
                        emit_oproj(*oproj_q.pop(0))

            def emit_oproj(tq, oc2):
                # out[tq, oc2] = O @ WoT + 0.5 b_o (partial over this core's
                # 512 W_o input columns)
                ps = psS.tile([128, 512], f32, tag="s",
                              name=f"op{tq}_{oc2}")
                for ct in range(OC // 128):
                    nc.tensor.matmul(
                        ps[:],
                        lhsT=OT[:, ct, tq * 128:(tq + 1) * 128],
                        rhs=WO[:, ct, oc2 * 512:(oc2 + 1) * 512],
                        start=(ct == 0), stop=(ct == OC // 128 - 1),
                    )
                ob = ostage.tile([128, 512], bf16, tag="ob")
                nc.vector.tensor_tensor(
                    out=ob[:], in0=ps[:],
                    in1=BO[:, oc2 * 512:(oc2 + 1) * 512],
                    op=mybir.AluOpType.add,
                )
                # alternate rings so the final drain parallelizes (NOT
                # scalar: DGE launches there stall the exp pipeline)
                eng = nc.sync if (tq + oc2) % 2 == 0 else nc.gpsimd
                eng.dma_start(
                    out=out[tq * 128:(tq + 1) * 128,
                            oc2 * 512:(oc2 + 1) * 512],
                    in_=ob[:],
                )

            # ---- attention per head; O.T accumulated with V' stationary ----
            # two heads (one even, one odd) are software-pipelined: while
            # ScalarE exps head A's scores, PE runs head B's score matmuls.
            def st_exp(h, J, pair):
                prow = (h % 2) * 64
                QTh = QKT[prow:prow + 64, h // 2, :]
                KTh = QKT[prow:prow + 64, 4 + h // 2, :]
                ps = psS.tile([128, 1024], f32, tag="s",
                              name=f"ps{h}_{J}_{pair[0]}")
                pt = ptp.tile([128, 1024], bf16, tag="p",
                              name=f"pt{h}_{J}_{pair[0]}")
                col = 0
                offs = []
                for i in pair:
                    qlo = max(J * 512, i * 128)
                    span = (J + 1) * 512 - qlo
                    # each matmul region must stay within one bank
                    assert col // 512 == (col + span - 1) // 512
                    nc.tensor.matmul(
                        ps[:, col:col + span],
                        lhsT=KTh[:, i * 128:(i + 1) * 128],
                        rhs=QTh[:, qlo:qlo + span],
                        start=True, stop=True,
                    )
                    offs.append((i, col, qlo, span))
                    col += span
                nc.scalar.activation(
                    out=pt[:, 0:col], in_=ps[:, 0:col], func=Exp,
                    scale=0.125 / (SX * SW) ** 2)
                return pt, offs

            def av(h, J, pt, offs, otr):
                for i, coff, qlo, span in offs:
                    if i >= 4 * J:  # diagonal: zero upper triangle
                        nc.gpsimd.tensor_tensor(
                            out=pt[:, coff:coff + 128],
                            in0=pt[:, coff:coff + 128], in1=TRI[:],
                            op=mybir.AluOpType.mult,
                        )
                    # O.T[:, qloc:512] += V'_i.T @ P.T_i
                    qloc = qlo - J * 512
                    nc.tensor.matmul(
                        otr[:, qloc:512],
                        lhsT=VP[:, i, h, :],
                        rhs=pt[:, coff:coff + span],
                        start=(i == 0), stop=(i == 4 * J + 3),
                    )

            def normalize(h, J, otr):
                # O.T rows (base prow) times 1/den rows (base drow; all 64
                # denominator rows are identical by construction)
                prow = (h % 2) * 64
                drow = 64 - prow
                rd = dnp.tile([128, 512], f32, tag="d", name=f"rd{h}_{J}")
                # reciprocal_approx_fast only works at partition base 0
                if drow == 0:
                    nc.vector.reciprocal_approx_fast(
                        rd[0:64, :], otr[0:64, :])
                else:
                    rdc = dnp.tile([128, 512], f32, tag="dc",
                                   name=f"rdc{h}_{J}")
                    nc.vector.tensor_copy(
                        rdc[0:64, :], otr[drow:drow + 64, :])
                    nc.vector.reciprocal_approx_fast(
                        rd[0:64, :], rdc[0:64, :])
                nc.vector.tensor_tensor(
                    out=OT[prow:prow + 64, h // 2, J * 512:(J + 1) * 512],
                    in0=otr[prow:prow + 64, :],
                    in1=rd[0:64, :],
                    op=mybir.AluOpType.mult,
                )

            for hp in range(HPC // 2):
                h0, h1 = 2 * hp, 2 * hp + 1
                for J in range(T // 512):
                    otr0 = psO.tile([128, 512], f32, tag="o",
                                    name=f"otr{h0}_{J}")
                    otr1 = psO.tile([128, 512], f32, tag="o",
                                    name=f"otr{h1}_{J}")
                    ks = list(range(4 * J + 4))
                    pairs = [ks[m:m + 2] for m in range(0, len(ks), 2)]
                    prev = None
                    for pair in pairs:
                        pt0, offs0 = st_exp(h0, J, pair)
                        pt1, offs1 = st_exp(h1, J, pair)
                        pop_fill()
                        if prev is not None:
                            av(h0, J, prev[0][0], prev[0][1], otr0)
                            av(h1, J, prev[1][0], prev[1][1], otr1)
                        prev = ((pt0, offs0), (pt1, offs1))
                    av(h0, J, prev[0][0], prev[0][1], otr0)
                    av(h1, J, prev[1][0], prev[1][1], otr1)
                    normalize(h0, J, otr0)
                    normalize(h1, J, otr1)
                    if hp == HPC // 2 - 1:
                        for tq in range(4 * J, 4 * J + 4):
                            for oc2 in range(D // 512):
                                oproj_q.append((tq, oc2))
            while oproj_q:
                emit_oproj(*oproj_q.pop(0))

            if debug:
                nc.sync.dma_start(
                    out=d_qkt[:, :], in_=QKT[:].rearrange("p a t -> p (a t)"))
                nc.sync.dma_start(
                    out=d_ot[:, :], in_=OT[:].rearrange("p a t -> p (a t)"))

    nc.compile()
    return nc


def _in_maps(x, W_qkv, b_qkv, W_o, b_o):
    x = np.asarray(x, np.float32)
    W_qkv = np.asarray(W_qkv, np.float32)
    b_qkv = np.asarray(b_qkv, np.float32)
    W_o = np.asarray(W_o, np.float32)
    b_o = np.asarray(b_o, np.float32)

    maps = []
    for c in range(N_CORES):
        b, hh = c // 2, c % 2
        rs = slice(hh * OC, (hh + 1) * OC)
        wq = W_qkv[0 * D:1 * D][rs]            # [512, 1024]
        wk = W_qkv[1 * D:2 * D][rs]
        wvv = W_qkv[2 * D:3 * D][rs]
        wqkT = np.concatenate([wq, wk], 0).T   # [1024 d, 1024 o]
        bq = b_qkv[0 * D:1 * D][rs]
        bk = b_qkv[1 * D:2 * D][rs]
        bvv = b_qkv[2 * D:3 * D][rs]
        tri = np.triu(np.ones((128, 128), np.float32))
        xt = np.ascontiguousarray(x[b].T)      # [D, T]

        def ptile(m):                          # [n*128, F] -> [128, n*F]
            n = m.shape[0] // 128
            return np.ascontiguousarray(
                m.reshape(n, 128, -1).transpose(1, 0, 2).reshape(128, -1))

        def tmajor(m, tw):                     # [n*128, T] -> [128, T//tw, n, tw]
            n = m.shape[0] // 128
            return np.ascontiguousarray(
                m.reshape(n, 128, -1, tw).transpose(1, 2, 0, 3)
                .reshape(128, -1))

        # W8 chunk-interleaved: [q0,k0,q1,k1,q2,k2,q3,k3] x [n, 128cols]
        w8m = (wqkT * SW).reshape(ND, 128, 8, 128)     # [n, p, chunk, j]
        w8m = w8m.transpose(1, 2, 0, 3)                # [p, chunk, n, j]
        w8m = w8m[:, [0, 4, 1, 5, 2, 6, 3, 7]].reshape(128, -1)
        maps.append({
            "xT": tmajor(xt, 128).astype(BF16),
            "x8": tmajor(xt * SX, 512).astype(E4M3),
            "w8": np.ascontiguousarray(w8m).astype(E4M3),
            "wvT": ptile(np.ascontiguousarray(wvv.T)).astype(BF16),
            "woT": ptile(np.ascontiguousarray(W_o[:, rs].T)).astype(BF16),
            "bqk": np.ascontiguousarray(
                (SX * SW) * np.concatenate([bq, bk])
                .reshape(2 * OC // 128, 128).T),
            "bv": bvv.reshape(1, OC),
            "bo": (0.5 * b_o).reshape(1, D),
            "tri": tri.astype(BF16),
        })
    return maps


def _run(x, W_qkv, b_qkv, W_o, b_o, trace=False, tmpdir=None):
    from concourse.bass_utils import run_bass_kernel_spmd

    if "nc" not in _cache:
        _cache["nc"] = _build()
    res = run_bass_kernel_spmd(
        _cache["nc"], _in_maps(x, W_qkv, b_qkv, W_o, b_o),
        core_ids=list(range(N_CORES)), trace=trace, tmpdir=tmpdir,
    )
    out = np.empty((B, T, D), np.float32)
    for b in range(B):
        out[b] = (res.results[2 * b]["out"].astype(np.float32)
                  + res.results[2 * b + 1]["out"].astype(np.float32))
    return out, res


def kernel(x, W_qkv, b_qkv, W_o, b_o):
    out, _ = _run(x, W_qkv, b_qkv, W_o, b_o, trace=False)
    return out


# revision 47
# speedup vs baseline: 1.0001x; 1.0001x over previous
"""Causal multi-head attention block on 8 Trainium2 NeuronCores.

Problem: x[4,2048,1024] -> QKV proj (16 heads, dh=64) -> causal softmax
attention -> out proj. Sharding: core = (batch, head-half): each core
computes QKV for 8 heads of one batch, flash-style attention for those
heads, and a partial O-projection over its 512 W_o input columns; the
host sums the two partials per batch (tensor-parallel unshard).

Device kernel (identical SPMD program, per-core data), matmuls bf16
with fp32 PSUM accumulation, EXCEPT the Q/K projection which runs in
fp8e4m3 with DoubleRow perf mode: contraction 1024 in 4 passes instead
of 8 (DoubleRow processes 2 k-subtiles per pass), a true 2x on the PE.
x and W_q/W_k are pre-scaled (x16 / x256) on the host so fp8
quantization stays clear of subnormals; Q.T/K.T are stored bf16 SCALED
by 4096 and the 1/4096^2 descale is folded into the exp's activation
scale. V projection, scores, P@V and O-projection stay bf16 (fp8
there fails the accuracy budget; fp8 DoubleRow on the K=64 score
matmuls gives no wall-clock win anyway - the win scales with
contraction-pass count).

  - x.T is host pre-transposed; Q.T/K.T computed in [o, t] feature-major
    layout, V in [t, o].
  - scores are computed transposed, S.T[k_tile, q_span] = K.T_blk^T@Q.T,
    two k-tiles packed side by side in one 2-bank PSUM tile so ScalarE
    exps them in a single ACTIVATE (descale folded in; scores are O(1)
    here so softmax needs no max-subtraction). Diagonal blocks are
    masked after exp with a 0/1 triangle multiply on the Pool engine
    (GpSimd), keeping DVE free for PSUM evictions.
  - O.T[c, q] accumulates with V' stationary: V' has 64 V columns and 64
    ones-columns (parity-dependent order), so each head's O.T lands on
    its final partition rows and the matmul broadcasts the softmax
    denominator into the other 64 rows for free. Normalization is then
    one reciprocal_approx_fast + one cross-partition-base multiply
    (odd heads read the denominator straight from PSUM partition 0,
    even heads need one repositioning copy), writing OT[c, t] directly.
  - two heads are software-pipelined (PE runs head B scores while
    ScalarE exps head A) and the P@V matmuls lag one iteration behind
    the exps; QKV-projection and O-projection units are paced into the
    attention phase as PE filler so the ScalarE-bound stretches keep the
    TensorE array busy (and the HAM clock-gate warm). A short burst of
    junk matmuls warms the PE while the input DMAs stream; inputs are
    DMA'd in consumption order across three rings so the first
    projection unit starts ~2us in.
"""

import numpy as np
import ml_dtypes

BF16 = ml_dtypes.bfloat16
E4M3 = ml_dtypes.float8_e4m3

B, T, D = 4, 2048, 1024
NH, DH = 16, 64
HPC = 8            # heads per core
OC = HPC * DH      # 512: per-core head columns
NT = T // 128      # 16 q/k tiles of 128
ND = D // 128      # 8 d-tiles
N_CORES = 8
SX = 16.0          # fp8 x pre-scale
SW = 256.0         # fp8 W_q/W_k pre-scale

_cache = {}


def _build(debug=False):
    import concourse.mybir as mybir
    import concourse.tile as tile
    from concourse import bacc

    f32 = mybir.dt.float32
    bf16 = mybir.dt.bfloat16
    f8 = mybir.dt.float8e4
    Exp = mybir.ActivationFunctionType.Exp
    DR = mybir.MatmulPerfMode.DoubleRow

    nc = bacc.Bacc("TRN2", target_bir_lowering=False, debug=False,
                   num_devices=N_CORES)

    # all large inputs are host-pre-tiled to [128, ...] partition-major
    # layout so every DMA is one contiguous multi-KB run per partition
    xT = nc.declare_dram_parameter("xT", [128, ND * T], bf16, isOutput=False)
    x8 = nc.declare_dram_parameter("x8", [128, ND * T], f8, isOutput=False)
    w8 = nc.declare_dram_parameter("w8", [128, ND * 2 * OC], f8, isOutput=False)
    wv = nc.declare_dram_parameter("wvT", [128, ND * OC], bf16, isOutput=False)
    wo = nc.declare_dram_parameter("woT", [128, (OC // 128) * D], bf16,
                                   isOutput=False)
    bqk = nc.declare_dram_parameter("bqk", [128, 2 * OC // 128], f32, isOutput=False)
    bv = nc.declare_dram_parameter("bv", [1, OC], f32, isOutput=False)
    bo = nc.declare_dram_parameter("bo", [1, D], f32, isOutput=False)
    tri = nc.declare_dram_parameter("tri", [128, 128], bf16, isOutput=False)
    # bf16 wire format: halves output DMA traffic; host upcasts and sums
    out = nc.declare_dram_parameter("out", [T, D], bf16, isOutput=True)
    if debug:
        d_qkt = nc.declare_dram_parameter("d_qkt", [128, ND * T], bf16, isOutput=True)
        d_ot = nc.declare_dram_parameter(
            "d_ot", [128, (OC // 128) * T], bf16, isOutput=True)

    with tile.TileContext(nc) as tc:
        with (
            tc.tile_pool(name="persist", bufs=1) as persist,
            tc.tile_pool(name="pt", bufs=8) as ptp,
            tc.tile_pool(name="dn", bufs=6) as dnp,
            tc.tile_pool(name="ostage", bufs=4) as ostage,
            tc.tile_pool(name="psS", bufs=3, space="PSUM") as psS,
            tc.tile_pool(name="psO", bufs=2, space="PSUM") as psO,
        ):
            # ---- persistent SBUF tensors ----
            # x.T in t-major layouts so DMA slices arrive in consumption
            # order as contiguous multi-KB runs: XT by 128-t-tile (for
            # emit_v lhsT), X8 by 512-t-quarter (for emit_qk rhs)
            XT = persist.tile([128, NT, ND, 128], bf16)
            X8 = persist.tile([128, 4, ND, 512], f8)
            # W_qk.T fp8, chunk-interleaved [q0,k0,q1,k1,q2,k2,q3,k3] so
            # the prologue pair (q0,k0) is one contiguous leading slice
            W8 = persist.tile([128, 8, ND, 128], f8)
            WV = persist.tile([128, ND, OC], bf16)
            WO = persist.tile([128, OC // 128, D], bf16)
            BQK = persist.tile([128, 2 * OC // 128], f32)
            BV = persist.tile([128, OC], f32)
            BO = persist.tile([128, D], f32)
            TRI = persist.tile([128, 128], bf16)
            QKT = persist.tile([128, ND, T], bf16)         # [o, t] Q.T|K.T
            # V' per head, 128 cols: even h: [V(64) | 1*64]; odd h:
            # [1*64 | V(64)]. O.T rows land on partitions (h%2)*64..+64 and
            # the other 64 rows all become the softmax denominator (the
            # matmul broadcasts it for free).
            VP = persist.tile([128, NT, HPC, 128], bf16)
            OT = persist.tile([128, OC // 128, T], bf16)   # attn out.T [c, t]

            # warm-up: keep PE busy (and the HAM un-throttled) while the
            # input DMAs stream in; results are never read.
            JNK = persist.tile([128, 512], bf16)
            nc.vector.memset(JNK[:], 0.5)
            # prepay the one-time exp table load while DMAs stream
            wrm = dnp.tile([128, 512], f32, tag="d", name="actwarm")
            nc.scalar.activation(out=wrm[:, 0:16], in_=JNK[:, 0:16],
                                 func=Exp, scale=1.0)
            for g in range(2):
                jps = psS.tile([128, 512], f32, tag="s", name=f"jnk{g}")
                for m in range(11):
                    nc.tensor.matmul(
                        jps[:], lhsT=JNK[:, 0:128], rhs=JNK[:],
                        start=(m == 0), stop=(m == 10),
                    )

            xTr = xT.rearrange("p (a n t) -> p a n t", a=NT, n=ND)
            x8r = x8.rearrange("p (a n t) -> p a n t", a=4, n=ND)
            w8r = w8.rearrange("p (s n o) -> p s n o", s=8, n=ND)
            wvr = wv.rearrange("p (n o) -> p n o", n=ND)
            wor = wo.rearrange("p (c o) -> p c o", c=OC // 128)

            # input DMAs: each launch engine's transfers SERIALIZE on its
            # ring at ~110 GB/s, so tensors are split across the three
            # rings in strict consumption order; host pre-tiling makes
            # every slice one contiguous multi-KB run per partition
            # (fragmented strided DMAs run at <20 GB/s).
            #  sync:   BQK, X8 q0, WV hi, TRI, X8 q2, XT tt8-15 (+outs)
            #  scalar: W8 (q0,k0), WV lo, X8 q1, X8 q3, W8 rest, WO
            #  gpsimd: XT tt0-3, BV, XT tt4-7, BO (+outs)
            nc.sync.dma_start(out=BQK[:], in_=bqk[:, :])
            nc.scalar.dma_start(out=W8[:, 0:2], in_=w8r[:, 0:2])
            nc.gpsimd.dma_start(out=XT[:, 0:4], in_=xTr[:, 0:4])
            nc.sync.dma_start(out=X8[:, 0], in_=x8r[:, 0])
            nc.scalar.dma_start(out=WV[:, 0:4], in_=wvr[:, 0:4])
            nc.gpsimd.dma_start(out=BV[:], in_=bv[:, :].to_broadcast((128, OC)))
            nc.sync.dma_start(out=WV[:, 4:8], in_=wvr[:, 4:8])
            nc.scalar.dma_start(out=X8[:, 1], in_=x8r[:, 1])
            nc.gpsimd.dma_start(out=XT[:, 4:8], in_=xTr[:, 4:8])
            nc.sync.dma_start(out=TRI[:], in_=tri[:, :])
            nc.sync.dma_start(out=X8[:, 2], in_=x8r[:, 2])
            nc.scalar.dma_start(out=X8[:, 3], in_=x8r[:, 3])
            nc.gpsimd.dma_start(out=BO[:], in_=bo[:, :].to_broadcast((128, D)))
            nc.sync.dma_start(out=XT[:, 8:12], in_=xTr[:, 8:12])
            nc.scalar.dma_start(out=W8[:, 2:4], in_=w8r[:, 2:4])
            nc.gpsimd.dma_start(out=XT[:, 12:16], in_=xTr[:, 12:16])
            nc.scalar.dma_start(out=W8[:, 4:8], in_=w8r[:, 4:8])
            nc.scalar.dma_start(out=WO[:, 0:2], in_=wor[:, 0:2])
            nc.scalar.dma_start(out=WO[:, 2:4], in_=wor[:, 2:4])
            # V' ones columns on the Pool engine (SBUF-only op; DVE stays
            # free for evictions)
            nc.gpsimd.memset(VP[:, :, 0:HPC:2, DH:128], 1.0)
            nc.gpsimd.memset(VP[:, :, 1:HPC:2, 0:DH], 1.0)

            # ---- QKV projection, emitted as fill-in units ----
            def emit_qk(ot, tch):
                # one [o, t] chunk: [128 o, 512 t] = W_qk @ x.T + b, in fp8
                # DoubleRow: 4 passes over (kd, kd+1) pairs instead of 8.
                # Result is scaled by SX*SW; bias is pre-scaled to match.
                ps = psS.tile([128, 512], f32, tag="s",
                              name=f"qk{ot}_{tch}")
                ws = 2 * (ot % 4) + (0 if ot < 4 else 1)
                for kd in range(0, ND, 2):
                    nc.tensor.matmul(
                        ps[:],
                        lhsT=W8[:, ws, kd:kd + 2, :],
                        rhs=X8[:, tch, kd:kd + 2, :],
                        start=(kd == 0), stop=(kd == ND - 2),
                        perf_mode=DR,
                    )
                nc.vector.tensor_scalar_add(
                    QKT[:, ot, tch * 512:(tch + 1) * 512], ps[:],
                    BQK[:, ot:ot + 1],
                )

            def emit_v(tt):
                # one [t, o] tile of V = x @ W_v.T + b, into parity layout
                ps = psS.tile([128, 512], f32, tag="s", name=f"v{tt}")
                for kd in range(ND):
                    nc.tensor.matmul(
                        ps[:],
                        lhsT=XT[:, tt, kd, :],
                        rhs=WV[:, kd, :],
                        start=(kd == 0), stop=(kd == ND - 1),
                    )
                nc.vector.tensor_tensor(
                    out=VP[:, tt, 0:HPC:2, 0:DH],
                    in0=ps[:].rearrange("p (a b) -> p a b", b=DH)[:, 0:HPC:2, :],
                    in1=BV[:].rearrange("p (a b) -> p a b", b=DH)[:, 0:HPC:2, :],
                    op=mybir.AluOpType.add,
                )
                nc.vector.tensor_tensor(
                    out=VP[:, tt, 1:HPC:2, DH:2 * DH],
                    in0=ps[:].rearrange("p (a b) -> p a b", b=DH)[:, 1:HPC:2, :],
                    in1=BV[:].rearrange("p (a b) -> p a b", b=DH)[:, 1:HPC:2, :],
                    op=mybir.AluOpType.add,
                )

            # prologue: only what head-pair 0's first iteration needs
            emit_qk(0, 0)
            emit_qk(4, 0)
            # the rest is interleaved into the attention phase as PE
            # filler. Deadlines (giter of the pop_fill that must emit):
            #   Q(m,t): 20m + t^2+t-1; K(4+m,t): 20m + t^2+3t-1
            #   V(i): J=i//4: J^2+J + min(i//2, 2J+1)  (same-giter OK: the
            #   avs of a pair run after that giter's pop_fill)
            sched = {
                0: [("v", 0), ("v", 1)],
                1: [("v", 2), ("v", 3), ("qk", 0, 1)],
                2: [("qk", 4, 1)],
                3: [("v", 4), ("v", 5)],
                4: [("qk", 0, 2), ("v", 6)],
                5: [("v", 7)],
                7: [("qk", 4, 2)],
                8: [("v", 8), ("v", 9)],
                9: [("qk", 0, 3)],
                10: [("v", 10), ("v", 11)],
                14: [("qk", 4, 3)],
                16: [("v", 12), ("v", 13)],
                17: [("v", 14), ("v", 15)],
                18: [("qk", 1, 0)],
                19: [("qk", 5, 0)],
                20: [("qk", 1, 1)],
                22: [("qk", 5, 1)],
                24: [("qk", 1, 2)],
                28: [("qk", 5, 2)],
                30: [("qk", 1, 3)],
                36: [("qk", 5, 3)],
                38: [("qk", 2, 0)],
                39: [("qk", 6, 0)],
                40: [("qk", 2, 1)],
                42: [("qk", 6, 1)],
                44: [("qk", 2, 2)],
                48: [("qk", 6, 2)],
                50: [("qk", 2, 3)],
                56: [("qk", 6, 3)],
                58: [("qk", 3, 0)],
                59: [("qk", 7, 0)],
                60: [("qk", 3, 1)],
                62: [("qk", 7, 1)],
                64: [("qk", 3, 2)],
                68: [("qk", 7, 2)],
                70: [("qk", 3, 3)],
                76: [("qk", 7, 3)],
            }
            giter = [0]
            oproj_q = []

            def pop_fill():
                g = giter[0]
                giter[0] += 1
                for u in sched.get(g, []):
                    if u[0] == "v":
                        emit_v(u[1])
                    else:
                        emit_qk(u[1], u[2])
                for _ in range(3):
                    if oproj_q:
                        emit_oproj(*oproj_q.pop(0))

            def emit_oproj(tq, oc2):
                # out[tq, oc2] = O @ WoT + 0.5 b_o (partial over this core's
                # 512 W_o input columns)
                ps = psS.tile([128, 512], f32, tag="s",
                              name=f"op{tq}_{oc2}")
                for ct in range(OC // 128):
                    nc.tensor.matmul(
                        ps[:],
                        lhsT=OT[:, ct, tq * 128:(tq + 1) * 128],
                        rhs=WO[:, ct, oc2 * 512:(oc2 + 1) * 512],
                        start=(ct == 0), stop=(ct == OC // 128 - 1),
                    )
                ob = ostage.tile([128, 512], bf16, tag="ob")
                nc.vector.tensor_tensor(
                    out=ob[:], in0=ps[:],
                    in1=BO[:, oc2 * 512:(oc2 + 1) * 512],
                    op=mybir.AluOpType.add,
                )
                # alternate rings so the final drain parallelizes (NOT
                # scalar: DGE launches there stall the exp pipeline)
                eng = nc.sync if (tq + oc2) % 2 == 0 else nc.gpsimd
                eng.dma_start(
                    out=out[tq * 128:(tq + 1) * 128,
                            oc2 * 512:(oc2 + 1) * 512],
                    in_=ob[:],
                )

            # ---- attention per head; O.T accumulated with V' stationary ----
            # two heads (one even, one odd) are software-pipelined: while
            # ScalarE exps head A's scores, PE runs head B's score matmuls.
            def st_exp(h, J, pair):
                prow = (h % 2) * 64
                QTh = QKT[prow:prow + 64, h // 2, :]
                KTh = QKT[prow:prow + 64, 4 + h // 2, :]
                ps = psS.tile([128, 1024], f32, tag="s",
                              name=f"ps{h}_{J}_{pair[0]}")
                pt = ptp.tile([128, 1024], bf16, tag="p",
                              name=f"pt{h}_{J}_{pair[0]}")
                col = 0
                offs = []
                for i in pair:
                    qlo = max(J * 512, i * 128)
                    span = (J + 1) * 512 - qlo
                    # each matmul region must stay within one bank
                    assert col // 512 == (col + span - 1) // 512
                    nc.tensor.matmul(
                        ps[:, col:col + span],
                        lhsT=KTh[:, i * 128:(i + 1) * 128],
                        rhs=QTh[:, qlo:qlo + span],
                        start=True, stop=True,
                    )
                    offs.append((i, col, qlo, span))
                    col += span
                nc.scalar.activation(
                    out=pt[:, 0:col], in_=ps[:, 0:col], func=Exp,
                    scale=0.125 / (SX * SW) ** 2)
                return pt, offs

            def av(h, J, pt, offs, otr):
                for i, coff, qlo, span in offs:
                    if i >= 4 * J:  # diagonal: zero upper triangle
                        nc.gpsimd.tensor_tensor(
                            out=pt[:, coff:coff + 128],
                            in0=pt[:, coff:coff + 128], in1=TRI[:],
                            op=mybir.AluOpType.mult,
                        )
                    # O.T[:, qloc:512] += V'_i.T @ P.T_i
                    qloc = qlo - J * 512
                    nc.tensor.matmul(
                        otr[:, qloc:512],
                        lhsT=VP[:, i, h, :],
                        rhs=pt[:, coff:coff + span],
                        start=(i == 0), stop=(i == 4 * J + 3),
                    )

            def normalize(h, J, otr):
                # O.T rows (base prow) times 1/den rows (base drow; all 64
                # denominator rows are identical by construction)
                prow = (h % 2) * 64
                drow = 64 - prow
                rd = dnp.tile([128, 512], f32, tag="d", name=f"rd{h}_{J}")
                # reciprocal_approx_fast only works at partition base 0
                if drow == 0:
                    nc.vector.reciprocal_approx_fast(
                        rd[0:64, :], otr[0:64, :])
                else:
                    rdc = dnp.tile([128, 512], f32, tag="dc",
                                   name=f"rdc{h}_{J}")
                    nc.vector.tensor_copy(
                        rdc[0:64, :], otr[drow:drow + 64, :])
                    nc.vector.reciprocal_approx_fast(
                        rd[0:64, :], rdc[0:64, :])
                nc.vector.tensor_tensor(
                    out=OT[prow:prow + 64, h // 2, J * 512:(J + 1) * 512],
                    in0=otr[prow:prow + 64, :],
                    in1=rd[0:64, :],
                    op=mybir.AluOpType.mult,
                )

            for hp in range(HPC // 2):
                h0, h1 = 2 * hp, 2 * hp + 1
                for J in range(T // 512):
                    otr0 = psO.tile([128, 512], f32, tag="o",
                                    name=f"otr{h0}_{J}")
                    otr1 = psO.tile([128, 512], f32, tag="o",
                                    name=f"otr{h1}_{J}")
                    ks = list(range(4 * J + 4))
                    pairs = [ks[m:m + 2] for m in range(0, len(ks), 2)]
                    prev = None
                    for pair in pairs:
                        pt0, offs0 = st_exp(h0, J, pair)
                        pt1, offs1 = st_exp(h1, J, pair)
                        pop_fill()
                        if prev is not None:
                            av(h0, J, prev[0][0], prev[0][1], otr0)
                            av(h1, J, prev[1][0], prev[1][1], otr1)
                        prev = ((pt0, offs0), (pt1, offs1))
                    av(h0, J, prev[0][0], prev[0][1], otr0)
                    av(h1, J, prev[1][0], prev[1][1], otr1)
                    normalize(h0, J, otr0)
                    normalize(h1, J, otr1)
                    if hp == HPC // 2 - 1:
                        for tq in range(4 * J, 4 * J + 4):
                            for oc2 in range(D // 512):
                                oproj_q.append((tq, oc2))
            while oproj_q:
                emit_oproj(*oproj_q.pop(0))

            if debug:
                nc.sync.dma_start(
                    out=d_qkt[:, :], in_=QKT[:].rearrange("p a t -> p (a t)"))
                nc.sync.dma_start(
                    out=d_ot[:, :], in_=OT[:].rearrange("p a t -> p (a t)"))

    nc.compile()
    return nc


def _in_maps(x, W_qkv, b_qkv, W_o, b_o):
    x = np.asarray(x, np.float32)
    W_qkv = np.asarray(W_qkv, np.float32)
    b_qkv = np.asarray(b_qkv, np.float32)
    W_o = np.asarray(W_o, np.float32)
    b_o = np.asarray(b_o, np.float32)

    maps = []
    for c in range(N_CORES):
        b, hh = c // 2, c % 2
        rs = slice(hh * OC, (hh + 1) * OC)
        wq = W_qkv[0 * D:1 * D][rs]            # [512, 1024]
        wk = W_qkv[1 * D:2 * D][rs]
        wvv = W_qkv[2 * D:3 * D][rs]
        wqkT = np.concatenate([wq, wk], 0).T   # [1024 d, 1024 o]
        bq = b_qkv[0 * D:1 * D][rs]
        bk = b_qkv[1 * D:2 * D][rs]
        bvv = b_qkv[2 * D:3 * D][rs]
        tri = np.triu(np.ones((128, 128), np.float32))
        xt = np.ascontiguousarray(x[b].T)      # [D, T]

        def ptile(m):                          # [n*128, F] -> [128, n*F]
            n = m.shape[0] // 128
            return np.ascontiguousarray(
                m.reshape(n, 128, -1).transpose(1, 0, 2).reshape(128, -1))

        def tmajor(m, tw):                     # [n*128, T] -> [128, T//tw, n, tw]
            n = m.shape[0] // 128
            return np.ascontiguousarray(
                m.reshape(n, 128, -1, tw).transpose(1, 2, 0, 3)
                .reshape(128, -1))

        # W8 chunk-interleaved: [q0,k0,q1,k1,q2,k2,q3,k3] x [n, 128cols]
        w8m = (wqkT * SW).reshape(ND, 128, 8, 128)     # [n, p, chunk, j]
        w8m = w8m.transpose(1, 2, 0, 3)                # [p, chunk, n, j]
        w8m = w8m[:, [0, 4, 1, 5, 2, 6, 3, 7]].reshape(128, -1)
        maps.append({
            "xT": tmajor(xt, 128).astype(BF16),
            "x8": tmajor(xt * SX, 512).astype(E4M3),
            "w8": np.ascontiguousarray(w8m).astype(E4M3),
            "wvT": ptile(np.ascontiguousarray(wvv.T)).astype(BF16),
            "woT": ptile(np.ascontiguousarray(W_o[:, rs].T)).astype(BF16),
            "bqk": np.ascontiguousarray(
                (SX * SW) * np.concatenate([bq, bk])
                .reshape(2 * OC // 128, 128).T),
            "bv": bvv.reshape(1, OC),
            "bo": (0.5 * b_o).reshape(1, D),
            "tri": tri.astype(BF16),
        })
    return maps


def _run(x, W_qkv, b_qkv, W_o, b_o, trace=False, tmpdir=None):
    from concourse.bass_utils import run_bass_kernel_spmd

    if "nc" not in _cache:
        _cache["nc"] = _build()
    res = run_bass_kernel_spmd(
        _cache["nc"], _in_maps(x, W_qkv, b_qkv, W_o, b_o),
        core_ids=list(range(N_CORES)), trace=trace, tmpdir=tmpdir,
    )
    out = np.empty((B, T, D), np.float32)
    for b in range(B):
        out[b] = (res.results[2 * b]["out"].astype(np.float32)
                  + res.results[2 * b + 1]["out"].astype(np.float32))
    return out, res


def kernel(x, W_qkv, b_qkv, W_o, b_o):
    out, _ = _run(x, W_qkv, b_qkv, W_o, b_o, trace=False)
    return out
